# revision 1
# baseline (speedup 1.0000x reference)
"""Multi-head attention with RoPE (B=4, N=2048, C=1024, H=16, d=64) on 8
Trainium2 NeuronCores.

Sharding: tensor-parallel over heads — each core computes 2 of the 16 heads
(Wq/Wkv sharded column-wise, Wout row-wise). Each core returns a partial
yT = (out_h @ Wout_h).T over the full batch; the host sums the 8 partials.

Per-core kernel (all matmuls bf16, fp32 PSUM accumulation):
  - x is pre-transposed on the host to xT [C, B*N] bf16 so the contraction
    dim (C) lands on SBUF partitions.
  - Projections run W-stationary, 4 token-chunks wide (weight loads amortized):
    psum [feat 128, tok 512] -> q^T/k^T with RoPE applied on evacuation
    (rotate-half via partition-swapped SBUF copy; sign folded into the
    host-prepared sin table), v^T cast to bf16.
  - v is transposed to token-major via a DRAM bounce + XBAR DMA transpose,
    with ones columns interleaved so the PV matmul (M=65) also produces the
    softmax denominators.
  - Attention per 512-query chunk: S^T tiles = k_tile.T@q (two heads packed
    in one PE pass via tile_position row groups), exp on ScalarE straight
    from PSUM (scale=1/8 folded in; no max-subtraction needed: |S*scale| < 3),
    PV accumulates O^T + denominator in PSUM over the 16 key tiles.
  - Normalize via fast reciprocal + gpsimd partition_broadcast; output
    projection produces y^T chunks which are DMA'd straight from SBUF.
"""

import numpy as np
import ml_dtypes
from contextlib import ExitStack

import concourse.bass as bass
import concourse.tile as tile
from concourse import bacc, mybir
from concourse.bass_utils import run_bass_kernel_spmd

P = 128
B, NSEQ, C = 4, 2048, 1024
H, D = 16, 64
NTOK = B * NSEQ
KO = C // P
QC = 512
NKT = NSEQ // P
NQC = NSEQ // QC
FC = C // P
VW = 160  # vtok row width: [v_h0 | 1 | v_h1 | 1 | pad] (32-multiple for XBAR)
NPRE = int(__import__("os").environ.get("NPRE", "3"))  # QK+exp groups of the next chunk prefetched before boundary bursts
BF = mybir.dt.bfloat16
F32 = mybir.dt.float32

NB = B


def _build():
    nc = bacc.Bacc("TRN2", target_bir_lowering=False, debug=False)

    xT = nc.dram_tensor("xT", [C, NTOK], BF, kind="ExternalInput").ap()
    wq = nc.dram_tensor("wq", [C, P], BF, kind="ExternalInput").ap()
    wk = nc.dram_tensor("wk", [C, P], BF, kind="ExternalInput").ap()
    wv = nc.dram_tensor("wv", [C, P], BF, kind="ExternalInput").ap()
    wout = nc.dram_tensor("wout", [P, C], BF, kind="ExternalInput").ap()
    cos2 = nc.dram_tensor("cos2", [P, NSEQ], F32, kind="ExternalInput").ap()
    sin2s = nc.dram_tensor("sin2s", [P, NSEQ], F32, kind="ExternalInput").ap()
    yT = nc.dram_tensor("yT", [C, NTOK], F32, kind="ExternalOutput").ap()

    with ExitStack() as ctx:
        tc = ctx.enter_context(tile.TileContext(nc))
        consts = ctx.enter_context(tc.tile_pool(name="consts", bufs=1))
        xpool = ctx.enter_context(tc.tile_pool(name="xpool", bufs=2))
        qkpool = ctx.enter_context(tc.tile_pool(name="qkpool", bufs=2))
        vpool = ctx.enter_context(tc.tile_pool(name="vpool", bufs=2))
        rope = ctx.enter_context(tc.tile_pool(name="rope", bufs=2))
        pexp_pool = ctx.enter_context(tc.tile_pool(name="pexp", bufs=11))
        onorm_pool = ctx.enter_context(tc.tile_pool(name="onorm", bufs=3))
        ytmp_pool = ctx.enter_context(tc.tile_pool(name="ytmp", bufs=3))
        small = ctx.enter_context(tc.tile_pool(name="small", bufs=2))
        dram = ctx.enter_context(tc.tile_pool(name="dram", bufs=2, space="DRAM"))
        ps_io = ctx.enter_context(tc.tile_pool(name="ps_io", bufs=2, space="PSUM"))
        ps_s = ctx.enter_context(tc.tile_pool(name="ps_s", bufs=2, space="PSUM"))
        ps_o = ctx.enter_context(tc.tile_pool(name="ps_o", bufs=2, space="PSUM"))

        # ---- constants ----
        wq_sb = consts.tile([P, KO, P], BF, tag="wq")
        wk_sb = consts.tile([P, KO, P], BF, tag="wk")
        wv_sb = consts.tile([P, KO, P], BF, tag="wv")
        wout_sb = consts.tile([P, FC, P], BF, tag="wout")
        cos_sb = consts.tile([P, NSEQ], F32, tag="cos")
        sin_sb = consts.tile([P, NSEQ], F32, tag="sin")
        nc.sync.dma_start(wq_sb[:], wq.rearrange("(ko p) f -> p ko f", p=P))
        nc.sync.dma_start(wk_sb[:], wk.rearrange("(ko p) f -> p ko f", p=P))
        nc.sync.dma_start(wv_sb[:], wv.rearrange("(ko p) f -> p ko f", p=P))
        nc.sync.dma_start(wout_sb[:], wout.rearrange("r (fc f) -> r fc f", f=P))
        nc.sync.dma_start(cos_sb[:], cos2)
        nc.sync.dma_start(sin_sb[:], sin2s)
        ones_row = consts.tile([1, NSEQ], BF, tag="ones_row")
        nc.vector.memset(ones_row[:], 1.0)
        ones_blk = consts.tile([32, NSEQ], BF, tag="ones_blk")
        nc.vector.memset(ones_blk[:], 1.0)
        vbounces = []
        for i in range(2):
            vb = dram.tile([VW, NSEQ], BF, tag="vbounce", name=f"vb{i}")
            nc.sync.dma_start(vb[2 * D + 2 : VW, :], ones_blk[: VW - 2 * D - 2, :])
            vbounces.append(vb)

        def emit_load(b):
            t0 = b * NSEQ
            xb = xpool.tile([P, KO, NSEQ], BF, tag="xb", name="xb")
            xr = xT[:, t0 : t0 + NSEQ].rearrange("(ko p) t -> p ko t", p=P)
            for ko in range(KO):
                nc.sync.dma_start(xb[:, ko, :], xr[:, ko, :])
            qTt = qkpool.tile([P, NSEQ], BF, tag="qT", name="qT")
            kTt = qkpool.tile([P, NSEQ], BF, tag="kT", name="kT")
            vTt = qkpool.tile([P, NSEQ], BF, tag="vT", name="vT")
            vtok = vpool.tile([P, NKT, VW], BF, tag="vtok", name="vtok")
            return dict(xb=xb, qT=qTt, kT=kTt, vT=vTt, vtok=vtok, b=b)

        def emit_proj_tail(st, f, dst, ps, tsl):
            if f < 2:
                raw = rope.tile([P, QC], F32, tag="raw", name="raw")
                swp = rope.tile([P, QC], F32, tag="swp", name="swp")
                qcs = rope.tile([P, QC], F32, tag="qcs", name="qcs")
                qss = rope.tile([P, QC], F32, tag="qss", name="qss")
                nc.scalar.copy(raw[:], ps[:])
                for blk in range(4):
                    src = (blk ^ 1) * 32
                    nc.sync.dma_start(
                        swp[blk * 32 : blk * 32 + 32, :], raw[src : src + 32, :]
                    )
                nc.vector.tensor_mul(qcs[:], raw[:], cos_sb[:, tsl])
                nc.gpsimd.tensor_mul(qss[:], swp[:], sin_sb[:, tsl])
                nc.vector.tensor_add(dst[:, tsl], qcs[:], qss[:])
            else:
                nc.scalar.copy(st["vT"][:, tsl], ps[:])

        PIECE_ORDER = [2, 0, 1]  # v first, then q, k

        def emit_proj_piece(st, piece):
            f = PIECE_ORDER[piece]
            w_sb, dst = [(wq_sb, st["qT"]), (wk_sb, st["kT"]), (wv_sb, st["vT"])][f]
            pswide = ps_s.tile([P, 2, QC], F32, tag="pss", name="ps_proj")
            pp_a = ps_io.tile([P, QC], F32, tag="pp", name="pp_a")
            pp_b = ps_io.tile([P, QC], F32, tag="pp", name="pp_b")
            chunks = [pswide[:, 0, :], pswide[:, 1, :], pp_a[:], pp_b[:]]
            for ko in range(KO):
                for t4 in range(4):
                    nc.tensor.matmul(
                        chunks[t4],
                        w_sb[:, ko, :],
                        st["xb"][:, ko, t4 * QC : (t4 + 1) * QC],
                        start=(ko == 0),
                        stop=(ko == KO - 1),
                        skip_group_check=True,
                    )
            for t4 in range(4):
                emit_proj_tail(st, f, dst, chunks[t4], slice(t4 * QC, (t4 + 1) * QC))

        def emit_vtrans(st):
            b, vT, vtok = st["b"], st["vT"], st["vtok"]
            vbounce = vbounces[b % 2]
            nc.sync.dma_start(vbounce[0:D, :], vT[0:D, :])
            nc.sync.dma_start(vbounce[D + 1 : 2 * D + 1, :], vT[D : 2 * D, :])
            nc.sync.dma_start(vbounce[D : D + 1, :], ones_row[:])
            nc.sync.dma_start(vbounce[2 * D + 1 : 2 * D + 2, :], ones_row[:])
            nc.sync.dma_start_transpose(vtok[:, :, :], vbounce[:, :])

        def emit_qk_exp(st, qc, kt):
            qTt, kTt = st["qT"], st["kT"]
            qsl = slice(qc * QC, (qc + 1) * QC)
            ksl = slice(kt * P, (kt + 1) * P)
            pss = ps_s.tile([P, 2, QC], F32, tag="pss", name="pss_g")
            pexp = pexp_pool.tile([P, 2, QC], BF, tag="pexp", name="pexp_g")
            nc.tensor.matmul(
                pss[:, 0, :], kTt[0:D, ksl], qTt[0:D, qsl],
                start=True, stop=True, tile_position=(0, 0), skip_group_check=True,
            )
            nc.tensor.matmul(
                pss[:, 1, :], kTt[D : 2 * D, ksl], qTt[D : 2 * D, qsl],
                start=True, stop=True, tile_position=(64, 0), skip_group_check=True,
            )
            nc.scalar.activation(
                pexp[:], pss[:], mybir.ActivationFunctionType.Exp, scale=0.125
            )
            return pexp

        def emit_attn_core(st, qc, pre):
            vtok = st["vtok"]
            po0 = ps_o.tile([D + 1, QC], F32, tag="po", name="po0")
            po1 = ps_o.tile([D + 1, QC], F32, tag="po", name="po1")
            for kt in range(NKT):
                pexp = pre[kt] if kt < len(pre) else emit_qk_exp(st, qc, kt)
                nc.tensor.matmul(
                    po0[:], vtok[:, kt, 0 : D + 1], pexp[:, 0, :],
                    start=(kt == 0), stop=(kt == NKT - 1), skip_group_check=True,
                )
                nc.tensor.matmul(
                    po1[:], vtok[:, kt, D + 1 : 2 * D + 2], pexp[:, 1, :],
                    start=(kt == 0), stop=(kt == NKT - 1), skip_group_check=True,
                )

            onorm = onorm_pool.tile([P, QC], BF, tag="onorm", name="onorm")
            r0 = small.tile([1, QC], F32, tag="r0", name="r0")
            r1 = small.tile([1, QC], F32, tag="r1", name="r1")
            bc0 = small.tile([D, QC], F32, tag="bc0", name="bc0")
            bc1 = small.tile([D, QC], F32, tag="bc1", name="bc1")
            rs = small.tile([1, QC], F32, tag="rs", name="rs")
            rs2 = small.tile([1, QC], F32, tag="rs2", name="rs2")
            nc.vector.tensor_copy(rs[:], po0[D : D + 1, :])
            nc.vector.tensor_copy(rs2[:], po1[D : D + 1, :])
            nc.vector.reciprocal_approx_fast(r0[:], rs[:])
            nc.vector.reciprocal_approx_fast(r1[:], rs2[:])
            nc.gpsimd.partition_broadcast(bc0[:], r0[:])
            nc.gpsimd.partition_broadcast(bc1[:], r1[:])
            nc.vector.tensor_mul(onorm[0:D, :], po0[0:D, :], bc0[:])
            nc.vector.tensor_mul(onorm[D : 2 * D, :], po1[0:D, :], bc1[:])
            return onorm

        def emit_outproj(st, qc, onorm):
            t0 = st["b"] * NSEQ
            for fc in range(FC):
                py = ps_io.tile([P, QC], F32, tag="pp", name="py")
                nc.tensor.matmul(
                    py[:], wout_sb[:, fc, :], onorm[:], start=True, stop=True
                )
                yt = ytmp_pool.tile([P, QC], F32, tag="yt", name="yt")
                nc.vector.tensor_copy(yt[:], py[:])
                nc.sync.dma_start(
                    yT[fc * P : (fc + 1) * P, t0 + qc * QC : t0 + (qc + 1) * QC],
                    yt[:],
                )

        # ---- pipelined emission ----
        states = [None] * (NB + 1)
        states[0] = emit_load(0)
        for piece in range(3):
            emit_proj_piece(states[0], piece)
            if piece == 0:
                emit_vtrans(states[0])
        states[1] = emit_load(1)

        sched = [(b, qc) for b in range(NB) for qc in range(NQC)]
        pre = []
        for idx, (b, qc) in enumerate(sched):
            st = states[b]
            onorm = emit_attn_core(st, qc, pre)
            pre = []
            if idx + 1 < len(sched):
                nb_, nqc = sched[idx + 1]
                for kt in range(NPRE):
                    pre.append(emit_qk_exp(states[nb_], nqc, kt))
            emit_outproj(st, qc, onorm)
            if b + 1 < NB:
                if qc < 3:
                    emit_proj_piece(states[b + 1], qc)
                    if qc == 0:
                        emit_vtrans(states[b + 1])
                if qc == 0 and b + 2 < NB:
                    states[b + 2] = emit_load(b + 2)

    nc.compile()
    return nc


def _host_inputs(x, cos, sin, Wq, Wkv, Wout):
    bf = ml_dtypes.bfloat16
    xT = np.ascontiguousarray(x.reshape(NTOK, C).T).astype(bf)
    cosT = cos.reshape(NSEQ, D).T.astype(np.float32)
    sinT = sin.reshape(NSEQ, D).T.astype(np.float32)
    sign = np.where(np.arange(D)[:, None] < D // 2, -1.0, 1.0).astype(np.float32)
    cos2 = np.ascontiguousarray(np.concatenate([cosT, cosT], 0))
    sin2s = np.ascontiguousarray(np.concatenate([sinT * sign, sinT * sign], 0))
    maps = []
    for core in range(8):
        c0 = core * P
        maps.append(
            {
                "xT": xT,
                "wq": np.ascontiguousarray(Wq[:, c0 : c0 + P]).astype(bf),
                "wk": np.ascontiguousarray(Wkv[:, c0 : c0 + P]).astype(bf),
                "wv": np.ascontiguousarray(Wkv[:, C + c0 : C + c0 + P]).astype(bf),
                "wout": np.ascontiguousarray(Wout[c0 : c0 + P, :]).astype(bf),
                "cos2": cos2,
                "sin2s": sin2s,
            }
        )
    return maps


_nc_cache = None


def _get_nc():
    global _nc_cache
    if _nc_cache is None:
        _nc_cache = _build()
    return _nc_cache


def kernel(x, cos, sin, Wq, Wkv, Wout, bout, _trace=False):
    x = np.asarray(x, dtype=np.float32)
    cos = np.asarray(cos, dtype=np.float32)
    sin = np.asarray(sin, dtype=np.float32)
    Wq = np.asarray(Wq, dtype=np.float32)
    Wkv = np.asarray(Wkv, dtype=np.float32)
    Wout = np.asarray(Wout, dtype=np.float32)
    bout = np.asarray(bout, dtype=np.float32)

    nc = _get_nc()
    in_maps = _host_inputs(x, cos, sin, Wq, Wkv, Wout)
    res = run_bass_kernel_spmd(nc, in_maps, list(range(8)), trace=_trace)

    y = np.zeros((C, NTOK), np.float32)
    for c in range(8):
        y += res.results[c]["yT"]
    out = y.T.reshape(B, NSEQ, C) + bout
    if _trace:
        return out, res
    return out

